# revision 31
# baseline (speedup 1.0000x reference)
"""Trainium2 Bass kernel for DigitConvolutionalModel.

Model: x[B,784] -> reshape [B,1,28,28] -> 3x3 valid conv (1 channel)
       -> flatten [B,676] -> relu(@w1[676,128] + b1) -> @w2[128,10] + b2.

Strategy (PE-bound after dtype compression; ~25 us/core):
  * Conv is linear, so fold it into fc1 during weight prep: W_eff[784,128] =
    C @ w1 where C[784,676] is the conv-as-matmul operator. The device
    computes relu(x @ W_eff + b1) @ w2 + b2 -- one 784-contraction matmul and
    one 128-contraction matmul over the full batch.
  * Pure data parallel over 8 NeuronCores: batch dim sharded 8 x 8192, tiny
    weights replicated.
  * fp8 streaming: x is cast to float8_e3m4 on the host; the PE accepts a
    mixed-dtype matmul (e3m4 moving x, fp16 stationary W_eff). Measured
    end-to-end rel err 1.25e-2 vs the 2e-2 gate (fp16 weights keep the
    W-side exact to ~3e-4; the x-side e3m4 rounding dominates). This cuts
    the HBM stream 4x vs fp32 (6.4 MB/core, ~19 us) so the kernel runs at
    the PE roofline instead: fc1 = ceil(784/128)=7 passes x 512 cols x 16
    blocks = 57k cycles ~= 24 us at the warm 2.4 GHz clock. The PE queue
    never drains, which also keeps the HAM activity throttle at 8/8.
  * Host lays x out feature-major AND group-blocked ([NGRP, 112, LB*7*512])
    so each load is ONE fully-contiguous ~0.8 MB DMA (7 KB per partition
    line) -- max DMA efficiency, no on-device transposes. Contraction is 7
    uniform chunks of K=112 (784 = 7*112), accumulating into a [128,512]
    PSUM bank.
  * bias+relu in ONE ACT-engine op (scale-free: out = relu(psum + b1)) into
    fp16 h; fc2 then uses h as the STATIONARY operand (4 sub-matmuls of
    just 10 moving columns each, weight-loads hidden by the PE's reorder
    window) instead of streaming 512 columns -- fc2 costs ~40 PE cycles per
    block instead of 512. b2 is added on DVE ([128,4,10] per block) into a
    per-repeat SBUF accumulator, stored fp16 (host upconverts) with a
    single 160 KB DMA per repeat.
  * PE queue scheduling: fc2(t-1) is emitted after fc1(t) ("defer") so the
    in-order PE queue never waits on the relu of the block it just
    produced.
  * Group loads alternate between the SP and ACT HWDGE rings; constants
    ride SWDGE (gpsimd).
"""

import sys

sys.path.insert(0, "/opt/trn_rl_repo")

import numpy as np

import concourse.bass as bass
import concourse.bacc as bacc
import concourse.mybir as mybir
import concourse.tile as tile
from concourse.bass_utils import run_bass_kernel_spmd

N_CORES = 8
B_FULL = 65536
B_CORE = B_FULL // N_CORES  # 8192
D_IN = 784  # 28*28
KC = 112  # contraction chunk (784 = 7*112)
NCH = 7
D_HID = 128
D_OUT = 10
BLK = 512  # batch block per fc1 matmul group (max moving free dim)
NBLK = B_CORE // BLK  # 16
LB = 2  # blocks per load group
HB = BLK // 2  # relu column half

_compiled = None
MODE = "f16"

NSUB = BLK // D_HID  # 4 batch sub-tiles per block for stat_h fc2

# default build knobs (shared by kernel() and test.py's measure_hw)
KNOBS = dict(lb=LB, defer=2, relu_mode="act", ldw_hoist=True,
             x_dtype="f8e3", fc2_mode="stat_h", out16=True)


def _build_weff(conv_w: np.ndarray, w1: np.ndarray) -> np.ndarray:
    """W_eff[784,128]: folded conv+fc1 weights (fp64 accumulation)."""
    w1v = w1.astype(np.float64).reshape(26, 26, D_HID)
    acc = np.zeros((28, 28, D_HID), dtype=np.float64)
    cw = conv_w.astype(np.float64)
    for dr in range(3):
        for dc in range(3):
            acc[dr : dr + 26, dc : dc + 26, :] += cw[dr, dc] * w1v
    return acc.reshape(D_IN, D_HID)


def _build_bass(xt_bufs=3, h_bufs=6, o_bufs=2, ph_bufs=2, po_bufs=3,
                repeat=1, mode="f16", lb=None, defer=None, relu_mode=None,
                ldw_hoist=None, x_dtype=None, stage=4, fc2_mode=None,
                out16=None):
    lb = KNOBS["lb"] if lb is None else lb
    defer = KNOBS["defer"] if defer is None else defer
    relu_mode = KNOBS["relu_mode"] if relu_mode is None else relu_mode
    ldw_hoist = KNOBS["ldw_hoist"] if ldw_hoist is None else ldw_hoist
    x_dtype = KNOBS["x_dtype"] if x_dtype is None else x_dtype
    fc2_mode = KNOBS["fc2_mode"] if fc2_mode is None else fc2_mode
    out16 = KNOBS["out16"] if out16 is None else out16
    odt = mybir.dt.float16 if out16 else mybir.dt.float32

    ngrp = NBLK // lb
    nc = bacc.Bacc("TRN2", target_bir_lowering=False, debug=False, num_devices=1)
    f32 = mybir.dt.float32
    f16 = mybir.dt.float16
    xdt = {"f16": f16, "f8e3": mybir.dt.float8e3,
           "f8e4": mybir.dt.float8e4}[x_dtype]

    xt_d = nc.dram_tensor("xt", [ngrp, KC, lb, NCH, BLK], xdt,
                          kind="ExternalInput").ap()
    w_d = nc.dram_tensor("w", [KC, NCH, D_HID], f16, kind="ExternalInput").ap()
    b1_d = nc.dram_tensor("b1", [D_HID], f32, kind="ExternalInput").ap()
    w2_d = nc.dram_tensor("w2", [D_HID, D_OUT], f16, kind="ExternalInput").ap()
    b2_d = nc.dram_tensor("b2", [D_OUT], f32, kind="ExternalInput").ap()
    if fc2_mode == "trans":
        out_d = nc.dram_tensor("out", [D_OUT, B_CORE], odt,
                               kind="ExternalOutput").ap()
    else:  # stat_h: batch-subtile-major [p, t, s, c]
        out_d = nc.dram_tensor("out", [D_HID, NBLK, NSUB, D_OUT], odt,
                               kind="ExternalOutput").ap()

    with tile.TileContext(nc) as tc:
        with (
            tc.tile_pool(name="const", bufs=1) as const_pool,
            tc.tile_pool(name="xt", bufs=xt_bufs) as xtpool,
            tc.tile_pool(name="h", bufs=h_bufs) as hpool,
            tc.tile_pool(name="o", bufs=o_bufs) as opool,
            tc.tile_pool(name="ph", bufs=ph_bufs, space="PSUM") as ps_h,
            tc.tile_pool(name="po", bufs=po_bufs, space="PSUM") as ps_o,
        ):
            # constants ride SWDGE (gpsimd); both HWDGE rings are reserved
            # for the x stream
            w_sb = const_pool.tile([KC, NCH, D_HID], f16)
            nc.gpsimd.dma_start(w_sb, w_d)
            b1_sb = const_pool.tile([D_HID, 1], f32)
            nc.gpsimd.dma_start(b1_sb, b1_d.rearrange("(h o) -> h o", o=1))
            w2_sb = const_pool.tile([D_HID, D_OUT], f16)
            nc.gpsimd.dma_start(w2_sb, w2_d)
            if fc2_mode == "trans":
                b2_sb = const_pool.tile([D_OUT, 1], f32)
                nc.gpsimd.dma_start(b2_sb, b2_d.rearrange("(c o) -> c o", o=1))
            else:
                # b2 broadcast to all 128 partitions x NSUB for [p, s, c] add
                b2_sb = const_pool.tile([D_HID, NSUB, D_OUT], f32)
                b2_bcast = bass.AP(
                    tensor=b2_d.tensor, offset=b2_d.offset,
                    ap=[[0, D_HID], [0, NSUB]] + list(b2_d.ap),
                )
                nc.gpsimd.dma_start(b2_sb, b2_bcast)

            def relu(h_sb, hps):
                if relu_mode == "split":
                    nc.vector.tensor_scalar(
                        h_sb[:, :HB], hps[:, :HB], b1_sb, 0.0,
                        mybir.AluOpType.add, mybir.AluOpType.max,
                    )
                    nc.scalar.activation(
                        h_sb[:, HB:], hps[:, HB:],
                        mybir.ActivationFunctionType.Relu, bias=b1_sb,
                    )
                elif relu_mode == "act":
                    nc.scalar.activation(
                        h_sb, hps,
                        mybir.ActivationFunctionType.Relu, bias=b1_sb,
                    )
                else:  # dve
                    nc.vector.tensor_scalar(
                        h_sb, hps, b1_sb, 0.0,
                        mybir.AluOpType.add, mybir.AluOpType.max,
                    )

            for r in range(repeat):
                if stage < 3:
                    o_all = None
                elif fc2_mode == "trans":
                    o_all = opool.tile([D_OUT, B_CORE], odt)
                else:
                    o_all = opool.tile([D_HID, NBLK, NSUB, D_OUT], odt)
                hs = {}

                def fc2(t):
                    h_sb = hs.pop(t)
                    if fc2_mode == "trans":
                        ops = ps_o.tile([D_OUT, BLK], f32)
                        nc.tensor.matmul(ops, w2_sb, h_sb, start=True,
                                         stop=True)
                        nc.vector.tensor_scalar(
                            o_all[:, t * BLK : (t + 1) * BLK], ops, b2_sb,
                            None, mybir.AluOpType.add,
                        )
                    else:
                        # stationary-h fc2: 4x 10-column matmuls; the h
                        # weight-loads hide under fc1 via PE's reorder window
                        ops = ps_o.tile([D_HID, NSUB, D_OUT], f32)
                        for s in range(NSUB):
                            nc.tensor.matmul(
                                ops[:, s, :],
                                h_sb[:, s * D_HID : (s + 1) * D_HID],
                                w2_sb, start=True, stop=True,
                            )
                        nc.vector.tensor_add(o_all[:, t], ops, b2_sb)

                for g in range(ngrp):
                    # alternate the two HWDGE rings (SP / ACT) per group
                    eng = nc.sync if g % 2 == 0 else nc.scalar
                    xt_g = xtpool.tile([KC, lb, NCH, BLK], xdt)
                    eng.dma_start(xt_g, xt_d[g])

                    if ldw_hoist:
                        if stage < 1:
                            continue
                        # chunk-outer: one stationary load serves all lb
                        # blocks of the group
                        hpss = [ps_h.tile([D_HID, BLK], f32, name=f"hps{l}")
                                for l in range(lb)]
                        for c in range(NCH):
                            for l in range(lb):
                                nc.tensor.matmul(
                                    hpss[l], w_sb[:, c, :], xt_g[:, l, c, :],
                                    start=(c == 0), stop=(c == NCH - 1),
                                )
                        if stage < 2:
                            continue
                        for l in range(lb):
                            t = g * lb + l
                            h_sb = hpool.tile([D_HID, BLK], f16)
                            relu(h_sb, hpss[l])
                            hs[t] = h_sb
                        if stage < 3:
                            continue
                        for l in range(lb):
                            tp = (g - defer) * lb + l
                            if tp >= 0 and tp in hs:
                                fc2(tp)
                    else:
                        for l in range(lb):
                            t = g * lb + l
                            if stage < 1:
                                continue
                            hps = ps_h.tile([D_HID, BLK], f32)
                            for c in range(NCH):
                                nc.tensor.matmul(
                                    hps, w_sb[:, c, :], xt_g[:, l, c, :],
                                    start=(c == 0), stop=(c == NCH - 1),
                                )
                            if stage < 2:
                                continue
                            h_sb = hpool.tile([D_HID, BLK], f16)
                            relu(h_sb, hps)
                            if stage < 3:
                                continue
                            hs[t] = h_sb
                            if t - defer >= 0:
                                fc2(t - defer)
                # drain deferred fc2s
                if stage >= 3:
                    for t in sorted(hs.keys()):
                        fc2(t)
                # one 328 KB store per repeat, ring alternates per repeat
                seng = nc.scalar if r % 2 == 0 else nc.sync
                if stage >= 3:
                    seng.dma_start(out_d, o_all)
                elif fc2_mode == "trans":
                    seng.dma_start(out_d[:, 0:1], b2_sb)
                else:
                    seng.dma_start(out_d[:, 0], b2_sb)

    nc.compile()
    return nc


def _get_compiled():
    global _compiled
    if _compiled is None:
        _compiled = _build_bass()
    return _compiled


def _np_x_dtype():
    if KNOBS["x_dtype"] == "f16":
        return np.float16
    return mybir.dt.np(
        {"f8e3": mybir.dt.float8e3, "f8e4": mybir.dt.float8e4}[
            KNOBS["x_dtype"]
        ]
    )


def _make_in_maps(x, conv_w, w1, b1, w2, b2):
    lb = KNOBS["lb"]
    ngrp = NBLK // lb
    w_eff = _build_weff(conv_w, w1)  # [784, 128] fp64
    # host layout [k, c, h] so SBUF chunk c is W_eff rows c*112..c*112+111
    w_h = np.ascontiguousarray(
        w_eff.reshape(NCH, KC, D_HID).transpose(1, 0, 2)
    ).astype(np.float16)
    w2_h = np.asarray(w2, dtype=np.float16)
    b1_h = np.asarray(b1, dtype=np.float32)
    b2_h = np.asarray(b2, dtype=np.float32)
    xdt = _np_x_dtype()

    xs = np.asarray(x, dtype=np.float32).reshape(N_CORES, B_CORE, D_IN)
    maps = []
    for i in range(N_CORES):
        # [g, k, l, c, b]: feature f = c*112+k, batch col = (g*lb+l)*512+b
        xt = xs[i].T.reshape(NCH, KC, ngrp, lb, BLK)
        xb = np.ascontiguousarray(
            xt.transpose(2, 1, 3, 0, 4)
        ).astype(xdt)
        maps.append({
            "xt": xb, "w": w_h, "b1": b1_h, "w2": w2_h, "b2": b2_h,
        })
    return maps


def _gather(res):
    if KNOBS["fc2_mode"] == "trans":
        # device layout [10, 8192] per core -> [B_CORE, 10]
        parts = [r["out"].astype(np.float32).T for r in res.results]
    else:
        # device layout [128, NBLK, NSUB, 10]: batch = t*512 + s*128 + p
        parts = [
            r["out"].astype(np.float32).transpose(1, 2, 0, 3).reshape(
                B_CORE, D_OUT
            )
            for r in res.results
        ]
    return np.ascontiguousarray(np.concatenate(parts, axis=0))


def kernel(x, conv_w, w1, b1, w2, b2, **run_kwargs):
    nc = _get_compiled()
    in_maps = _make_in_maps(x, conv_w, w1, b1, w2, b2)
    res = run_bass_kernel_spmd(nc, in_maps, core_ids=list(range(N_CORES)),
                               **run_kwargs)
    out = _gather(res)
    if run_kwargs:
        return out, res
    return out
